# revision 19
# baseline (speedup 1.0000x reference)
"""Class-align loss (segment_reduce) Trainium2 kernel — v2.

Full inputs: f_source [4,256,128,128] f32, f_convert [4,256,128,128] f32,
seg [4,128,128] int32 (values in [0,19)). Output: scalar f32 triplet loss.

Strategy (data-parallel over batch*h-half, 8 shards):
  - Each core processes a [256, 8192] shard of each feature tensor.
    SWDGE cast fp32 -> bf16 in flight (as v1).
  - PE transposes [128c,128p] blocks into PSUM batch tiles [128p, 4x256c];
    ACT drains PSUM -> SBUF xT chunks (frees DVE).
  - Per compute chunk (8-16 groups): DVE squares (bf16 2x) + 3-level
    pairwise-add tree + one small tensor_reduce -> ssq[p, g]; ACT sqrt;
    DVE reciprocal -> r[p, g] = 1/||x_p||.
  - Class masks m[p, k, g] = (seg[p,g]==k) are built ONCE from a single
    broadcast is_equal; per chunk ONE broadcast tensor_tensor folds r:
    w[p, k, g] = m * r.
  - ONE class matmul per group: acc_t[19, 256] += w[:, :, g].T @ xT[:, g, :]
    (19-col stationary -> cheap ldweights; 256-col moving stream).
    PE total = transposes + class streams ~= 47us ~= DMA floor.
  - Host sums the 8 partial [2, 19, 256] outputs and runs the tiny
    triplet-loss epilogue in float64.

Walrus encodes at most ONE sync wait per instruction: staging tiles are
dedicated (wait-free DMAs), absorber transposes take the staging-DMA
waits on PE, sync=False ordering edges keep the PE stream near program
order, and the kernel-tail drain is split across sequencer NOPs.
"""

import sys

import numpy as np

if "/opt/trn_rl_repo" not in sys.path:
    sys.path.insert(0, "/opt/trn_rl_repo")

import concourse.bass as bass
import concourse.mybir as mybir
import concourse.tile as tile
from concourse.bass_utils import run_bass_kernel_spmd
from concourse.tile import add_dep_helper
from concourse.vector_clock import ScopedClock


def _split_drain_and_barrier(self, tick_clock, wait_clock):
    """Tile's kernel-tail drain carries one wait per semaphore the kernel
    ever used; split the excess onto dedicated sequencer NOPs (the 1-wait
    walrus encoding limit)."""
    nc = self.nc
    drain_inst = nc.sync.drain()
    wait_clock.add_sem_waits(
        drain_inst.ins, ScopedClock({None: tick_clock.global_clock})
    )
    si = drain_inst.ins.sync_info
    if si is not None and len(si.on_wait) > 1:
        waits = list(si.on_wait)
        upds = list(si.on_update)
        drain_inst.ins.sync_info = mybir.SyncInfo(
            on_wait=waits[:1], on_update=upds)
        for k in range(1, len(waits)):
            nop = nc.sync.nop(nofuse=True, hint=f"drain_wait_{k}")
            nop.ins.sync_info = mybir.SyncInfo(
                on_wait=[waits[k]], on_update=[])
    nc.all_engine_barrier()
    assert self.sems is not None
    popped = nc._tile_sem_poison_stack.pop()
    assert popped is self._sem_poison
    nc.clear_and_free_semaphores(list(self.sems.allocated().values()))
    nc.all_engine_barrier()


tile.TileContext._drain_and_barrier = _split_drain_and_barrier

def _fix_multiwait(nc):
    """Walrus encodes at most ONE sync wait per instruction. Tile's wait
    assigner occasionally leaves two (pool-rotation hazards whose transitive
    subsumption it misses). Move each excess wait onto an earlier same-engine
    instruction with a free wait slot. The moved wait's producer precedes the
    holder in the scheduler's linearization, so this cannot deadlock — it only
    makes the queue stall (harmlessly) a little earlier."""
    allins = []
    for f in nc.m.functions:
        for b in f.blocks:
            allins.extend(b.instructions)
    pos = {id(ins): i for i, ins in enumerate(allins)}
    # Map (sem, cumulative-count) -> producer position.
    cnt = {}
    reach = {}
    for i, ins in enumerate(allins):
        si = ins.sync_info
        if not si:
            continue
        for u in si.on_update:
            nm = u.ant_name
            cnt[nm] = cnt.get(nm, 0) + 1
            reach[(nm, cnt[nm])] = i

    def target_pos(w):
        # First position whose cumulative count for this sem >= wait_value.
        p = reach.get((w.ant_name, w.wait_value))
        return p if p is not None else -1

    ins_at = {i: ins for i, ins in enumerate(allins)}

    # Synchronous compute: queue-order retirement implies the sem bump has
    # been issued. Async ops (DMA) bump at transfer completion — not droppable.
    SYNC_TYPES = {
        "InstMatmult", "InstLdweights", "InstActivation", "InstTensorTensor",
        "InstTensorScalarPtr", "InstTensorCopy", "InstTensorReduce",
        "InstReciprocal", "InstStreamTranspose", "InstTensorTensorReduce",
        "InstEventSemaphore", "InstMemSet", "InstCopy", "InstIota",
    }

    dropped = 0
    for f in nc.m.functions:
        for b in f.blocks:
            for ins in b.instructions:
                si = ins.sync_info
                if not si or len(si.on_wait) <= 1:
                    continue
                mypos = pos[id(ins)]
                keep, droppable = [], []
                for w in si.on_wait:
                    tp = target_pos(w)
                    tgt = ins_at.get(tp)
                    if (tgt is not None and tgt.engine == ins.engine
                            and tp < mypos
                            and type(tgt).__name__ in SYNC_TYPES):
                        droppable.append(w)
                    else:
                        keep.append(w)
                while len(keep) < 1 and droppable:
                    keep.append(droppable.pop())
                if len(keep) > 1:
                    raise RuntimeError(
                        f"{ins.name} still has {len(keep)} real waits: " +
                        str([(w.ant_name, w.wait_value) for w in keep]))
                dropped += len(droppable)
                ins.sync_info = mybir.SyncInfo(
                    on_wait=keep, on_update=list(si.on_update))
    return dropped


# Problem constants (hardcoded; kernel.py must be self-contained).
B, C, H, W = 4, 256, 128, 128
N_CLASS = 19
N_CORES = 8
EPS_NORM = 1e-12
EPS_TRIP = 1e-6
MARGIN = 0.2

P = 128                      # SBUF partitions / pixel-group size
NPIX = B * H * W // N_CORES  # 8192 pixels per core
NG = NPIX // P               # 64 pixel groups per core

_NC_CACHE = {}


def build_nc():
    f32 = mybir.dt.float32
    bf16 = mybir.dt.bfloat16
    i32 = mybir.dt.int32
    nc = bass.Bass()

    fs_dram = nc.declare_dram_parameter("f_source", [C, NPIX], f32, isOutput=False)
    aux_dram = nc.declare_dram_parameter("aux", [P, P + N_CLASS], f32,
                                         isOutput=False)
    fc_dram = nc.declare_dram_parameter("f_convert", [C, NPIX], f32, isOutput=False)
    seg_dram = nc.declare_dram_parameter("seg", [NPIX], i32, isOutput=False)
    out_dram = nc.declare_dram_parameter("out", [2, N_CLASS, C], f32,
                                         isOutput=True)

    with tile.TileContext(nc) as tc:
        with (
            tc.tile_pool(name="const", bufs=1) as const_pool,
            tc.tile_pool(name="stage", bufs=1) as stage_pool,
            tc.tile_pool(name="xt", bufs=5) as xt_pool,
            tc.tile_pool(name="work", bufs=4) as work_pool,
            tc.tile_pool(name="wpool", bufs=8) as w_pool,
            tc.tile_pool(name="psum_t", bufs=5, space="PSUM") as psum_t_pool,
            tc.tile_pool(name="psum_abs", bufs=1, space="PSUM") as psum_abs_pool,
            tc.tile_pool(name="psum_acc", bufs=1, space="PSUM") as psum_acc_pool,
        ):
            # identity + iota row arrive via DMA (the "aux" input).
            aux_sb = const_pool.tile([P, P + N_CLASS], f32, tag="aux")
            nc.sync.dma_start(out=aux_sb[:], in_=aux_dram[:])
            iota19 = aux_sb[:, P:P + N_CLASS]
            ident_bf = const_pool.tile([P, P], bf16, tag="ident_bf")
            nc.vector.tensor_copy(ident_bf[:], aux_sb[:, 0:P])
            identity = ident_bf[:]
            iota_bf = const_pool.tile([P, N_CLASS], bf16, tag="iota_bf")
            nc.vector.tensor_copy(iota_bf[:], iota19)

            # seg wanted as [pixel-within-group (partition), group (free)]:
            # load contiguously and PE-transpose (gather DMA would need 8192
            # descriptors).
            seg_i = const_pool.tile([NG, P], i32, tag="seg_i")
            nc.sync.dma_start(
                out=seg_i[:], in_=seg_dram[:].rearrange("(g p) -> g p", p=P))
            seg_f = const_pool.tile([NG, P], f32, tag="seg_f")
            nc.vector.tensor_copy(seg_f[:], seg_i[:])
            ident_sm = const_pool.tile([NG, NG], f32, tag="ident_sm")
            nc.vector.tensor_copy(ident_sm[:], aux_sb[:NG, :NG])
            seg_ps = psum_t_pool.tile([P, NG], f32, tag="pt", name="seg_ps",
                                      padded_shape=[P, 512])
            nc.tensor.transpose(seg_ps[:], seg_f[:], ident_sm[:])
            seg_bf = const_pool.tile([P, NG], bf16, tag="seg_bf")
            nc.vector.tensor_copy(seg_bf[:], seg_ps[:])

            # Class masks m[p, g, k] = (seg[p, g] == k), built once via one
            # broadcast is_equal (bf16 holds 0/1 and small ints exactly).
            # k contiguous so the class-matmul ldweights reads are packed.
            m_all = const_pool.tile([P, NG, N_CLASS], bf16, tag="m_all")
            nc.vector.tensor_tensor(
                out=m_all[:],
                in0=seg_bf[:].unsqueeze(2).broadcast_to((P, NG, N_CLASS)),
                in1=iota_bf[:].unsqueeze(1).broadcast_to((P, NG, N_CLASS)),
                op=mybir.AluOpType.is_equal)

            # Warm-up transpose: pre-syncs PE against ident_bf (DVE).
            warm = psum_t_pool.tile([P, P], bf16, tag="pt", name="warm",
                                    padded_shape=[P, 1024])
            nc.tensor.transpose(warm[:, 0:P], identity, identity)

            # Transposed class-sum accumulators acc[k, c] (fp32 PSUM).
            # SEPARATE banks: a matmul's start=True resets the accumulation
            # state of the whole bank, so two interleaved accumulation
            # groups must not share one.
            acc_tiles = {
                t: psum_acc_pool.tile([N_CLASS, C], f32,
                                      tag=f"acc_{t}", name=f"acc_{t}")
                for t in ("s", "c")
            }
            acc = {t: acc_tiles[t][:] for t in ("s", "c")}
            drams = {"s": fs_dram, "c": fc_dram}

            # Dedicated bank for the DMA-wait absorber transposes.
            absorb = psum_abs_pool.tile([P, 8 * P], bf16, tag="absorb",
                                        name="absorb", padded_shape=[P, 1024])

            mm_all = []
            pending = None

            def order_after_mm(inst, back=24):
                if len(mm_all) >= back:
                    add_dep_helper(inst.ins, mm_all[-back].ins, sync=False,
                                   reason="keep PE stream near program order")

            out_sb = const_pool.tile([N_CLASS, 2 * C], f32, tag="out_sb")

            def emit_mms(t, w, xT, g0, G):
                # Absorber transposes: take the ACT (xT copies) and DVE
                # (w fold) waits on PE so each class matmul carries at
                # most one sync wait (walrus limit).
                axt = nc.tensor.transpose(
                    absorb[:, 2 * P:3 * P], xT[:, G - 1, 0:P], identity)
                order_after_mm(axt)
                aw = nc.tensor.transpose(
                    absorb[0:N_CLASS, 3 * P:4 * P], w[:, 0, :], identity)
                order_after_mm(aw)
                # One class matmul per group:
                # acc[k, c] += w[:, g, :].T @ xT[:, g, :].
                for g in range(G):
                    gg = g0 + g
                    mm = nc.tensor.matmul(
                        acc[t],
                        lhsT=w[:, g, :],
                        rhs=xT[:, g, :],
                        start=(gg == 0), stop=(gg == NG - 1))
                    add_dep_helper(mm.ins, axt.ins, sync=False,
                                   reason="mm after xT absorber")
                    add_dep_helper(mm.ins, aw.ins, sync=False,
                                   reason="mm after w absorber")
                    mm_all.append(mm)
                if g0 + G == NG:
                    # This tensor's accumulation just closed: ship it now so
                    # the store overlaps the other tensor's remaining work.
                    j = 0 if t == "s" else 1
                    nc.vector.tensor_copy(
                        out_sb[:, j * C:(j + 1) * C], acc[t])
                    nc.sync.dma_start(out=out_dram[j],
                                      in_=out_sb[:, j * C:(j + 1) * C])

            # Load chunks (pixels): small first so compute ramps during
            # descriptor generation; 2048 thereafter so the tail chunks land
            # staggered and their norm chains overlap instead of serializing.
            chunks = [(0, 512), (512, 512), (1024, 1024), (2048, 2048),
                      (4096, 2048), (6144, 2048)]
            gbase = {"s": 0, "c": 0}
            for ci, (pix0, cpix) in enumerate(chunks):
                for t in ("s", "c"):
                    lo = stage_pool.tile([P, cpix], bf16,
                                         tag=f"{t}_lo_{ci}", name=f"{t}_lo_{ci}")
                    hi = stage_pool.tile([P, cpix], bf16,
                                         tag=f"{t}_hi_{ci}", name=f"{t}_hi_{ci}")
                    d1 = nc.gpsimd.dma_start(
                        out=lo[:], in_=drams[t][0:P, pix0:pix0 + cpix])
                    d2 = nc.gpsimd.dma_start(
                        out=hi[:], in_=drams[t][P:C, pix0:pix0 + cpix])
                    if ci >= 1:
                        order_after_mm(d1, back=48)
                        order_after_mm(d2, back=48)
                    ab1 = nc.tensor.transpose(absorb[:, 0:P], lo[:, 0:P],
                                              identity)
                    ab2 = nc.tensor.transpose(absorb[:, P:2 * P], hi[:, 0:P],
                                              identity)
                    order_after_mm(ab1)
                    order_after_mm(ab2)

                    if True:
                        G = cpix // P          # groups in this chunk
                        NB = G // 4            # psum batches (4 groups each)
                        xT = xt_pool.tile([P, 16, C], bf16, tag="xT", bufs=6)
                        sq = work_pool.tile([P, 16, C], bf16, tag="sq", bufs=3)
                        for bi in range(NB):
                            psumT = psum_t_pool.tile([P, 4 * C], bf16,
                                                     tag="pt",
                                                     padded_shape=[P, 1024])
                            tps = []
                            for g in range(4):
                                px = (bi * 4 + g) * P
                                t1 = nc.tensor.transpose(
                                    psumT[:, g * C:g * C + P],
                                    lo[:, px:px + P], identity)
                                t2 = nc.tensor.transpose(
                                    psumT[:, g * C + P:(g + 1) * C],
                                    hi[:, px:px + P], identity)
                                tps.extend((t1, t2))
                            for tp in tps:
                                order_after_mm(tp)
                            # PSUM -> SBUF drain on ACT (keeps DVE free).
                            nc.scalar.copy(
                                xT[:, bi * 4:(bi + 1) * 4, :], psumT[:])
                            # Per-batch squares right after the copy: off
                            # the chunk's critical chain. Every 4th batch on
                            # ACT (Square activation) to balance DVE/ACT.
                            if bi % 4 == 3:
                                nc.scalar.square(
                                    sq[:, bi * 4:(bi + 1) * 4, :],
                                    xT[:, bi * 4:(bi + 1) * 4, :])
                            else:
                                nc.vector.tensor_mul(
                                    sq[:, bi * 4:(bi + 1) * 4, :],
                                    xT[:, bi * 4:(bi + 1) * 4, :],
                                    xT[:, bi * 4:(bi + 1) * 4, :])

                        # ssq[p, g] = sum_c xT^2: 3-level pairwise tree +
                        # one small reduce.
                        t128 = work_pool.tile([P, 16, 128], bf16, tag="t128", bufs=3)
                        nc.vector.tensor_add(
                            t128[:, :G, :], sq[:, :G, 0:128], sq[:, :G, 128:256])
                        t64 = work_pool.tile([P, 16, 64], bf16, tag="t64", bufs=3)
                        nc.vector.tensor_add(
                            t64[:, :G, :], t128[:, :G, 0:64], t128[:, :G, 64:128])
                        t32 = work_pool.tile([P, 16, 32], bf16, tag="t32", bufs=4)
                        nc.vector.tensor_add(
                            t32[:, :G, :], t64[:, :G, 0:32], t64[:, :G, 32:64])
                        ssq = work_pool.tile([P, 16], f32, tag="ssq", bufs=8)
                        nc.vector.tensor_reduce(
                            out=ssq[:, :G], in_=t32[:, :G, :],
                            axis=mybir.AxisListType.X, op=mybir.AluOpType.add)
                        nrm = work_pool.tile([P, 16], f32, tag="nrm", bufs=8)
                        nc.scalar.sqrt(nrm[:, :G], ssq[:, :G])
                        r = work_pool.tile([P, 16], f32, tag="r", bufs=8)
                        nc.vector.reciprocal(r[:, :G], nrm[:, :G])

                        # Fold r into the class masks for this chunk.
                        g0 = gbase[t]
                        w = w_pool.tile([P, 16, N_CLASS], bf16, tag="w", bufs=8)
                        nc.vector.tensor_mul(
                            w[:, :G, :],
                            m_all[:, g0:g0 + G, :],
                            r[:, :G].unsqueeze(2).broadcast_to(
                                (P, G, N_CLASS)))

                        # Software-pipeline the class matmuls one chunk
                        # behind: the PE stream runs the NEXT chunk's
                        # transposes while this chunk's norm chain computes.
                        if pending is not None:
                            emit_mms(*pending)
                        pending = (t, w, xT, g0, G)
                        gbase[t] += G

            if pending is not None:
                emit_mms(*pending)
                pending = None

    _fix_multiwait(nc)
    return nc


def aux_array():
    ident = np.eye(P, dtype=np.float32)
    iota = np.tile(np.arange(N_CLASS, dtype=np.float32), (P, 1))
    return np.ascontiguousarray(np.concatenate([ident, iota], axis=1))


def shard_inputs(f_source, f_convert, seg):
    """Split by (batch, h-half) into 8 per-core input maps."""
    in_maps = []
    hh = H // 2
    aux = aux_array()
    for core in range(N_CORES):
        b, half = divmod(core, 2)
        h0 = half * hh
        in_maps.append({
            "f_source": np.ascontiguousarray(
                f_source[b, :, h0:h0 + hh, :]).reshape(C, NPIX),
            "f_convert": np.ascontiguousarray(
                f_convert[b, :, h0:h0 + hh, :]).reshape(C, NPIX),
            "seg": np.ascontiguousarray(seg[b, h0:h0 + hh, :]).reshape(NPIX),
            "aux": aux,
        })
    return in_maps


def epilogue(S, Csum):
    """Tiny triplet-loss tail on [19,256] class sums (float64 host math)."""
    n = float(B * H * W)
    cs = S.astype(np.float64) / n
    cc = Csum.astype(np.float64) / n
    cs = cs / np.maximum(np.linalg.norm(cs, axis=1, keepdims=True), EPS_NORM)
    cc = cc / np.maximum(np.linalg.norm(cc, axis=1, keepdims=True), EPS_NORM)
    D = np.linalg.norm(cs[:, None, :] - cc[None, :, :] + EPS_TRIP, axis=2)
    d_ap = np.diag(D)
    terms = np.maximum(d_ap[:, None] - D + MARGIN, 0.0)
    mask = 1.0 - np.eye(N_CLASS)
    loss = (terms * mask).sum() / (N_CLASS * (N_CLASS - 1))
    return np.float32(loss)


def kernel(f_source, f_convert, seg):
    if "nc" not in _NC_CACHE:
        _NC_CACHE["nc"] = build_nc()
    nc = _NC_CACHE["nc"]
    in_maps = shard_inputs(f_source, f_convert, seg)
    res = run_bass_kernel_spmd(nc, in_maps, core_ids=list(range(N_CORES)))
    S = np.zeros((N_CLASS, C), dtype=np.float64)
    Csum = np.zeros((N_CLASS, C), dtype=np.float64)
    for r in res.results:
        S += r["out"][0].astype(np.float64)
        Csum += r["out"][1].astype(np.float64)
    return epilogue(S, Csum)


if __name__ == "__main__":
    rng = np.random.default_rng(0)
    fs = rng.standard_normal((B, C, H, W), dtype=np.float32)
    fc = rng.standard_normal((B, C, H, W), dtype=np.float32)
    sg = rng.integers(0, N_CLASS, size=(B, H, W), dtype=np.int32)
    print(kernel(fs, fc, sg))
